# revision 1
# baseline (speedup 1.0000x reference)
"""Luong attention Bass/Tile kernel for Trainium2, data-parallel over batch
across 8 NeuronCores.

Per core (2 batches): keys^T = W^T @ enc^T (fp32r matmuls), score = dec @ keys^T,
softmax over keys axis, context = P @ enc.  All matmuls run in float32r
(TF32-grade, full PE rate).  The Dense bias is dropped: it adds a
per-query-row constant to the scores, which softmax cancels exactly, so
neither output depends on it.
"""
import numpy as np
from contextlib import ExitStack

import concourse.bass as bass
from concourse import bacc
import concourse.tile as tile
import concourse.mybir as mybir
from concourse.bass_utils import run_bass_kernel_spmd
from concourse.masks import make_identity

F32 = mybir.dt.float32
F32R = mybir.dt.float32r
AF = mybir.ActivationFunctionType
AX = mybir.AxisListType

B, TQ, TK, D = 16, 2048, 2048, 1024
NCORES = 8
BPC = B // NCORES            # batches per core
P = 128
DT = D // P                  # 8 d-tiles
QT = TQ // P                 # 16 query tiles per batch
KT = TK // P                 # 16 key tiles per batch
SW = 512                     # score chunk width (1 PSUM bank of fp32)
SC = TK // SW                # 4 score chunks per q tile
KCW = 256                    # keys-phase tk chunk width
KC = TK // KCW               # 8 keys chunks

_CACHE = {}


def _build():
    nc = bacc.Bacc("TRN2", target_bir_lowering=False, debug=False,
                   num_devices=NCORES)
    dec = nc.dram_tensor("dec", [BPC, TQ, D], F32R, kind="ExternalInput").ap()
    enc = nc.dram_tensor("enc", [BPC, TK, D], F32R, kind="ExternalInput").ap()
    w = nc.dram_tensor("w", [D, D], F32R, kind="ExternalInput").ap()
    ctx_out = nc.dram_tensor("ctx", [BPC, TQ, D], F32, kind="ExternalOutput").ap()
    align_out = nc.dram_tensor("align", [BPC, TQ, TK], F32,
                               kind="ExternalOutput").ap()

    with tile.TileContext(nc) as tc, ExitStack() as ctx:
        constp = ctx.enter_context(tc.tile_pool(name="const", bufs=1))
        keysp = ctx.enter_context(tc.tile_pool(name="keys", bufs=1))
        statp = ctx.enter_context(tc.tile_pool(name="stat", bufs=2))
        psp = ctx.enter_context(tc.tile_pool(name="ps", bufs=6, space="PSUM"))
        tpp = ctx.enter_context(tc.tile_pool(name="tp", bufs=2, space="PSUM"))

        ident32 = constp.tile([P, P], F32)
        make_identity(nc, ident32[:])
        identr = constp.tile([P, P], F32R)
        nc.vector.tensor_copy(identr[:], ident32[:])

        for b in range(BPC):
            # ---------------- keys phase ----------------
            keysT = keysp.tile([P, DT, TK], F32R, tag="keysT")
            with tc.tile_pool(name=f"kph{b}", bufs=2) as kph, \
                 tc.tile_pool(name=f"wp{b}", bufs=1) as wp:
                w_t = wp.tile([P, DT, D], F32R, tag="w")
                nc.sync.dma_start(
                    w_t[:], w.rearrange("(di p) o -> p di o", p=P))
                for c in range(KC):
                    est = kph.tile([P, 2, D], F32R, tag="estream")
                    nc.sync.dma_start(
                        est[:],
                        enc[b, c * KCW:(c + 1) * KCW, :]
                        .rearrange("(j p) d -> p j d", p=P))
                    etile = kph.tile([P, DT, KCW], F32R, tag="encT")
                    for g in range(DT // 2):   # groups of 2 d-tiles
                        pst = tpp.tile([P, 4 * P], F32R, tag="tp")
                        for k in range(4):
                            di, j = g * 2 + k // 2, k % 2
                            nc.tensor.transpose(
                                pst[:, k * P:(k + 1) * P],
                                est[:, j, di * P:(di + 1) * P],
                                identr[:])
                        nc.any.tensor_copy(
                            etile[:, g * 2:g * 2 + 2, :], pst[:])
                    for do in range(DT):
                        kps = psp.tile([P, KCW], F32, tag="ps")
                        for di in range(DT):
                            nc.tensor.matmul(
                                kps[:], w_t[:, di, do * P:(do + 1) * P],
                                etile[:, di, :],
                                start=(di == 0), stop=(di == DT - 1))
                        nc.any.tensor_copy(
                            keysT[:, do, c * KCW:(c + 1) * KCW], kps[:])

            # ---------------- attention phase ----------------
            with tc.tile_pool(name=f"aph{b}", bufs=2) as aph, \
                 tc.tile_pool(name=f"encn{b}", bufs=1) as encnp:
                enc_nat = encnp.tile([P, KT, D], F32R, tag="encn")
                nc.sync.dma_start(
                    enc_nat[:], enc[b].rearrange("(t p) d -> p t d", p=P))

                state = [None] * QT  # per-tile softmax state for stage T

                def stage_s(i):
                    dnat = aph.tile([P, D], F32R, tag="dnat")
                    nc.sync.dma_start(dnat[:], dec[b, i * P:(i + 1) * P, :])
                    dt_t = aph.tile([P, DT, P], F32R, tag="decT")
                    for g in range(2):
                        pst = tpp.tile([P, 4 * P], F32R, tag="tp")
                        for k in range(4):
                            di = g * 4 + k
                            nc.tensor.transpose(
                                pst[:, k * P:(k + 1) * P],
                                dnat[:, di * P:(di + 1) * P], identr[:])
                        nc.any.tensor_copy(dt_t[:, g * 4:g * 4 + 4, :], pst[:])
                    mx = statp.tile([P, SC], F32, tag="mx")
                    scs = []
                    for n in range(SC):
                        sc_ps = psp.tile([P, SW], F32, tag="ps")
                        for di in range(DT):
                            nc.tensor.matmul(
                                sc_ps[:], dt_t[:, di, :],
                                keysT[:, di, n * SW:(n + 1) * SW],
                                start=(di == 0), stop=(di == DT - 1))
                        nc.vector.reduce_max(mx[:, n:n + 1], sc_ps[:], axis=AX.X)
                        scs.append(sc_ps)
                    negmax = statp.tile([P, 1], F32, tag="negmax")
                    nc.vector.reduce_max(negmax[:], mx[:], axis=AX.X, negate=True)
                    sums = statp.tile([P, SC], F32, tag="sums")
                    pt_un = aph.tile([P, TK], F32, tag="p_un")
                    for n in range(SC):
                        nc.scalar.activation(
                            pt_un[:, n * SW:(n + 1) * SW], scs[n][:], AF.Exp,
                            bias=negmax[:], accum_out=sums[:, n:n + 1])
                    ssum = statp.tile([P, 1], F32, tag="ssum")
                    nc.vector.reduce_sum(ssum[:], sums[:], axis=AX.X)
                    recip = statp.tile([P, 1], F32, tag="recip")
                    nc.vector.reciprocal(recip[:], ssum[:])
                    state[i] = (pt_un, recip)

                def stage_t(i):
                    pt_un, recip = state[i]
                    ptT = aph.tile([P, TK], F32R, tag="ptT")
                    for g in range(4):
                        pst = tpp.tile([P, 4 * P], F32, tag="tp")
                        for k in range(4):
                            t = g * 4 + k
                            nc.tensor.transpose(
                                pst[:, k * P:(k + 1) * P],
                                pt_un[:, t * P:(t + 1) * P], ident32[:])
                        nc.any.tensor_copy(
                            ptT[:, g * 4 * P:(g + 1) * 4 * P], pst[:])
                    cx = aph.tile([P, D], F32, tag="cx")
                    for dch in range(2):
                        cps = psp.tile([P, SW], F32, tag="ps")
                        for t in range(KT):
                            nc.tensor.matmul(
                                cps[:], ptT[:, t * P:(t + 1) * P],
                                enc_nat[:, t, dch * SW:(dch + 1) * SW],
                                start=(t == 0), stop=(t == KT - 1))
                        nc.vector.tensor_scalar_mul(
                            cx[:, dch * SW:(dch + 1) * SW], cps[:], recip[:])
                    # normalize alignment in place, then store both outputs
                    nc.vector.tensor_scalar_mul(pt_un[:], pt_un[:], recip[:])
                    nc.sync.dma_start(
                        align_out[b, i * P:(i + 1) * P, :], pt_un[:])
                    nc.sync.dma_start(ctx_out[b, i * P:(i + 1) * P, :], cx[:])
                    state[i] = None

                # 1-stage software pipeline: keep PE busy during softmax(i)
                stage_s(0)
                for i in range(1, QT):
                    stage_s(i)
                    stage_t(i - 1)
                stage_t(QT - 1)

    nc.compile()
    return nc


def kernel(decoder_output, encoder_output, wa_kernel, wa_bias):
    if "nc" not in _CACHE:
        _CACHE["nc"] = _build()
    nc = _CACHE["nc"]

    dec = np.ascontiguousarray(decoder_output, dtype=np.float32)
    enc = np.ascontiguousarray(encoder_output, dtype=np.float32)
    w = np.ascontiguousarray(wa_kernel, dtype=np.float32)

    in_maps = []
    for c in range(NCORES):
        lo, hi = c * BPC, (c + 1) * BPC
        in_maps.append({"dec": dec[lo:hi], "enc": enc[lo:hi], "w": w})

    res = run_bass_kernel_spmd(nc, in_maps, core_ids=list(range(NCORES)))
    context = np.concatenate([r["ctx"] for r in res.results], axis=0)
    alignment = np.concatenate([r["align"] for r in res.results], axis=0)
    return (context, alignment)


# revision 7
# speedup vs baseline: 34955.0556x; 34955.0556x over previous
"""Luong attention Bass/Tile kernel for Trainium2, data-parallel over batch
across 8 NeuronCores.

Per core (2 batches): keys^T = W^T @ enc^T (fp32r matmuls), score = dec @ keys^T,
softmax over keys axis, context = P @ enc.  All matmuls run in float32r
(TF32-grade, full PE rate).  The Dense bias is dropped: it adds a
per-query-row constant to the scores, which softmax cancels exactly, so
neither output depends on it.
"""
import numpy as np
from contextlib import ExitStack

import concourse.bass as bass
from concourse import bacc
import concourse.tile as tile
import concourse.mybir as mybir
from concourse.bass_utils import run_bass_kernel_spmd
from concourse.masks import make_identity

F32 = mybir.dt.float32
F32R = mybir.dt.float32r
AF = mybir.ActivationFunctionType
AX = mybir.AxisListType

B, TQ, TK, D = 16, 2048, 2048, 1024
NCORES = 8
BPC = B // NCORES            # batches per core
P = 128
DT = D // P                  # 8 d-tiles
QT = TQ // P                 # 16 query tiles per batch
KT = TK // P                 # 16 key tiles per batch
SW = 512                     # score chunk width (1 PSUM bank of fp32)
SC = TK // SW                # 4 score chunks per q tile
KCW = 256                    # keys-phase tk chunk width
KC = TK // KCW               # 8 keys chunks

_CACHE = {}


def _build():
    nc = bacc.Bacc("TRN2", target_bir_lowering=False, debug=False,
                   num_devices=NCORES)
    dec = nc.dram_tensor("dec", [BPC, TQ, D], F32R, kind="ExternalInput").ap()
    enc = nc.dram_tensor("enc", [BPC, TK, D], F32R, kind="ExternalInput").ap()
    w = nc.dram_tensor("w", [D, D], F32R, kind="ExternalInput").ap()
    ctx_out = nc.dram_tensor("ctx", [BPC, TQ, D], F32, kind="ExternalOutput").ap()
    align_out = nc.dram_tensor("align", [BPC, TQ, TK], F32,
                               kind="ExternalOutput").ap()

    with tile.TileContext(nc) as tc, ExitStack() as ctx:
        constp = ctx.enter_context(tc.tile_pool(name="const", bufs=1))
        keysp = ctx.enter_context(tc.tile_pool(name="keys", bufs=1))
        statp = ctx.enter_context(tc.tile_pool(name="stat", bufs=2))
        psp = ctx.enter_context(tc.tile_pool(name="ps", bufs=6, space="PSUM"))
        tpp = ctx.enter_context(tc.tile_pool(name="tp", bufs=2, space="PSUM"))

        ident32 = constp.tile([P, P], F32)
        make_identity(nc, ident32[:])
        identr = constp.tile([P, P], F32R)
        nc.vector.tensor_copy(identr[:], ident32[:])

        for b in range(BPC):
            # enc_nat + dec-tile pools span both phases: the keys phase DMAs
            # enc straight into enc_nat (the transpose source layout is the
            # same), so attention never waits on an enc load.
            with tc.tile_pool(name=f"encn{b}", bufs=1) as encnp, \
                 tc.tile_pool(name=f"decp{b}", bufs=2) as decp:
                enc_nat = encnp.tile([P, KT, D], F32R, tag="encn")
                enc_re = enc[b].rearrange("(t p) d -> p t d", p=P)

                # ---------------- keys phase ----------------
                keysT = keysp.tile([P, DT, TK], F32R, tag="keysT")
                with tc.tile_pool(name=f"kph{b}", bufs=2) as kph, \
                     tc.tile_pool(name=f"wp{b}", bufs=1) as wp:
                    # first enc chunk before W: transposes unblock first
                    nc.sync.dma_start(enc_nat[:, 0:2, :], enc_re[:, 0:2, :])
                    # W sliced per d_in tile: first matmuls only wait on
                    # slice 0; 8 DMAs spread across queues.
                    w_t = wp.tile([P, DT, D], F32R, tag="w")
                    wre = w.rearrange("(di p) o -> p di o", p=P)
                    for di in range(DT):
                        nc.sync.dma_start(w_t[:, di:di + 1, :],
                                          wre[:, di:di + 1, :])
                    # prefetch first dec tile for attention stage 0
                    dnat0 = decp.tile([P, D], F32R, tag="dnat")
                    nc.sync.dma_start(dnat0[:], dec[b, 0:P, :])
                    for c in range(KC):
                        if c > 0:
                            nc.sync.dma_start(
                                enc_nat[:, 2 * c:2 * c + 2, :],
                                enc_re[:, 2 * c:2 * c + 2, :])
                        etile = kph.tile([P, DT, KCW], F32R, tag="encT")
                        for g in range(DT // 2):   # groups of 2 d-tiles
                            pst = tpp.tile([P, 4 * P], F32R, tag="tp")
                            for k in range(4):
                                di, j = g * 2 + k // 2, k % 2
                                nc.tensor.transpose(
                                    pst[:, k * P:(k + 1) * P],
                                    enc_nat[:, 2 * c + j, di * P:(di + 1) * P],
                                    identr[:])
                            nc.any.tensor_copy(
                                etile[:, g * 2:g * 2 + 2, :], pst[:])
                        for do in range(DT):
                            kps = psp.tile([P, KCW], F32, tag="ps")
                            for di in range(DT):
                                nc.tensor.matmul(
                                    kps[:], w_t[:, di, do * P:(do + 1) * P],
                                    etile[:, di, :],
                                    start=(di == 0), stop=(di == DT - 1))
                            nc.any.tensor_copy(
                                keysT[:, do, c * KCW:(c + 1) * KCW], kps[:])

                # ---------------- attention phase ----------------
                aph_stack = ExitStack()
                aph = aph_stack.enter_context(
                    tc.tile_pool(name=f"aph{b}", bufs=2))

                state = [None] * QT  # per-tile softmax state for stage T

                def stage_s(i):
                    if i == 0:
                        dnat = dnat0
                    else:
                        dnat = decp.tile([P, D], F32R, tag="dnat")
                        nc.sync.dma_start(dnat[:],
                                          dec[b, i * P:(i + 1) * P, :])
                    dt_t = aph.tile([P, DT, P], F32R, tag="decT")
                    for g in range(2):
                        pst = tpp.tile([P, 4 * P], F32R, tag="tp")
                        for k in range(4):
                            di = g * 4 + k
                            nc.tensor.transpose(
                                pst[:, k * P:(k + 1) * P],
                                dnat[:, di * P:(di + 1) * P], identr[:])
                        nc.any.tensor_copy(dt_t[:, g * 4:g * 4 + 4, :], pst[:])
                    mx = statp.tile([P, SC], F32, tag="mx")
                    scs = []
                    for n in range(SC):
                        sc_ps = psp.tile([P, SW], F32, tag="ps")
                        for di in range(DT):
                            nc.tensor.matmul(
                                sc_ps[:], dt_t[:, di, :],
                                keysT[:, di, n * SW:(n + 1) * SW],
                                start=(di == 0), stop=(di == DT - 1))
                        nc.vector.reduce_max(mx[:, n:n + 1], sc_ps[:], axis=AX.X)
                        scs.append(sc_ps)
                    negmax = statp.tile([P, 1], F32, tag="negmax")
                    nc.vector.reduce_max(negmax[:], mx[:], axis=AX.X, negate=True)
                    sums = statp.tile([P, SC], F32, tag="sums")
                    pt_un = aph.tile([P, TK], F32, tag="p_un")
                    for n in range(SC):
                        nc.scalar.activation(
                            pt_un[:, n * SW:(n + 1) * SW], scs[n][:], AF.Exp,
                            bias=negmax[:], accum_out=sums[:, n:n + 1])
                    ssum = statp.tile([P, 1], F32, tag="ssum")
                    nc.vector.reduce_sum(ssum[:], sums[:], axis=AX.X)
                    recip = statp.tile([P, 1], F32, tag="recip")
                    nc.vector.reciprocal(recip[:], ssum[:])
                    state[i] = (pt_un, recip)

                def stage_t(i):
                    pt_un, recip = state[i]
                    ptT = aph.tile([P, TK], F32R, tag="ptT")
                    for g in range(4):
                        pst = tpp.tile([P, 4 * P], F32, tag="tp")
                        for k in range(4):
                            t = g * 4 + k
                            nc.tensor.transpose(
                                pst[:, k * P:(k + 1) * P],
                                pt_un[:, t * P:(t + 1) * P], ident32[:])
                        nc.any.tensor_copy(
                            ptT[:, g * 4 * P:(g + 1) * 4 * P], pst[:])
                    cx = aph.tile([P, D], F32, tag="cx")
                    for dch in range(2):
                        cps = psp.tile([P, SW], F32, tag="ps")
                        for t in range(KT):
                            nc.tensor.matmul(
                                cps[:], ptT[:, t * P:(t + 1) * P],
                                enc_nat[:, t, dch * SW:(dch + 1) * SW],
                                start=(t == 0), stop=(t == KT - 1))
                        nc.vector.tensor_scalar_mul(
                            cx[:, dch * SW:(dch + 1) * SW], cps[:], recip[:])
                    # normalize alignment in place, then store both outputs
                    nc.vector.tensor_scalar_mul(pt_un[:], pt_un[:], recip[:])
                    nc.sync.dma_start(
                        align_out[b, i * P:(i + 1) * P, :], pt_un[:])
                    nc.sync.dma_start(ctx_out[b, i * P:(i + 1) * P, :], cx[:])
                    state[i] = None

                # 1-stage software pipeline: keep PE busy during softmax(i).
                # enc_nat is already resident (loaded during the keys phase).
                stage_s(0)
                for i in range(1, QT):
                    stage_s(i)
                    stage_t(i - 1)
                stage_t(QT - 1)
                aph_stack.close()

    nc.compile()
    return nc


def kernel(decoder_output, encoder_output, wa_kernel, wa_bias):
    if "nc" not in _CACHE:
        _CACHE["nc"] = _build()
    nc = _CACHE["nc"]

    dec = np.ascontiguousarray(decoder_output, dtype=np.float32)
    enc = np.ascontiguousarray(encoder_output, dtype=np.float32)
    w = np.ascontiguousarray(wa_kernel, dtype=np.float32)

    in_maps = []
    for c in range(NCORES):
        lo, hi = c * BPC, (c + 1) * BPC
        in_maps.append({"dec": dec[lo:hi], "enc": enc[lo:hi], "w": w})

    res = run_bass_kernel_spmd(nc, in_maps, core_ids=list(range(NCORES)))
    context = np.concatenate([r["ctx"] for r in res.results], axis=0)
    alignment = np.concatenate([r["align"] for r in res.results], axis=0)
    return (context, alignment)
